# revision 17
# baseline (speedup 1.0000x reference)
"""GCN layer (message passing) on 8 trn2 NeuronCores.

  out = relu(segment_sum(norm * (H@W.T + b)[col], row)),  norm = d^-1/2[row] d^-1/2[col]
  with self-loops appended; d = 1 + in-degree.

Strategy (SPMD over 8 cores, edges partitioned by destination on host):
  - Aggregate-then-transform (GCN linearity):
      out[r] = relu(dis[r] * (Z[r] @ W.T + sigma[r] * b)),
      Z[r] = sum_{e: dst=r} dis[col_e] * H[col_e],  sigma[r] = sum_e dis[col_e]
    (sums include the self-loop edge r->r).
  - Host: shard nodes contiguously (12500/core), bin-pack each core's nodes
    into 98 blocks of 128 balancing per-block message counts; lay out each
    block's messages as CPB chunks of 128 edge slots; ship pre-scaled source
    features Hexp[e] = dis[col]*H[col] (bf16) in chunk-major [slot, chunk*D]
    layout plus per-slot dest keys dkT, per-node sigT/disT, W^T, bias.
  - Device per chunk: S = (iota == dk) one-hot (DVE); zt += Hexp_chunk^T @ S
    (PE, PSUM accum over the block's CPB chunks).  Per block: ztb = bf16(zt)
    (ACT); acc = sigma*bias (DVE preload) + ztb^T... acc += ztb.T @ W^T via
    PE (start=False); out_block = relu(acc * dis) (ACT) -> DMA.
  - No collectives, no dma_gather: GPSIMD stays idle; all DMA is bulk
    contiguous HWDGE.
"""
import numpy as np

N = 100000
D = 128
NCORES = 8
P = 128
NPC_REAL = N // NCORES          # 12500 real nodes per core
NBLK = 98                       # blocks of 128 slots (12544 slots, 44 dummy)
NPC = NBLK * P                  # 12544 slots per core


# ----------------------------------------------------------------- host prep

def _pack_blocks(deg_n, cpb):
    """Bin-pack all N nodes into 784 bins (<=128 each) balancing deg sums.

    deg_n: [N] message counts (in-degree + 1).  Returns [N] bin ids with
    every bin's deg sum <= cpb*128, or None if not achievable.
    """
    cap = cpb * P
    nbins = NCORES * NBLK
    order = np.argsort(-deg_n, kind="stable")
    bins = -np.ones(len(deg_n), dtype=np.int64)
    # snake placement of descending degrees
    for r in range(0, len(order), nbins):
        chunk = order[r:r + nbins]
        ids = np.arange(len(chunk))
        if (r // nbins) % 2 == 1:
            ids = nbins - 1 - ids
        bins[chunk] = ids[:len(chunk)]
    sums = np.bincount(bins, weights=deg_n, minlength=nbins).astype(np.int64)
    cnts = np.bincount(bins, minlength=nbins)
    # greedy fixup: swap items between fullest and emptiest bins
    for _ in range(20000):
        a = int(np.argmax(sums))
        if sums[a] <= cap:
            break
        b = int(np.argmin(sums))
        need = int(sums[a] - cap)
        room = int(cap - sums[b])
        ia = np.where(bins == a)[0]
        ib = np.where(bins == b)[0]
        da, db = deg_n[ia], deg_n[ib]
        # swap (i from a, j from b): diff = da_i - db_j must be >=1 and
        # <= room; prefer the smallest diff >= need, else the largest <= room
        diff = da[:, None] - db[None, :]
        ok = (diff >= 1) & (diff <= room)
        if not ok.any():
            return None
        dd = np.where(ok, diff, -1)
        best = np.where((dd >= min(need, room)) & ok, dd,
                        np.iinfo(np.int64).max)
        if best.min() != np.iinfo(np.int64).max:
            i, j = np.unravel_index(np.argmin(best), best.shape)
        else:
            i, j = np.unravel_index(np.argmax(dd), dd.shape)
        d = int(diff[i, j])
        bins[ia[i]], bins[ib[j]] = b, a
        sums[a] -= d
        sums[b] += d
    if sums.max() > cap or cnts.max() > P:
        return None
    return bins


def _host_prep(H, edge_index, W, b, CPB):
    """Build per-core device inputs; None if CPB chunks/block don't fit."""
    import ml_dtypes
    f32 = np.float32
    bf16 = ml_dtypes.bfloat16
    NC = NBLK * CPB                  # chunks per core

    row = np.asarray(edge_index[0], dtype=np.int64)
    col = np.asarray(edge_index[1], dtype=np.int64)
    H = np.asarray(H, dtype=f32)
    W = np.asarray(W, dtype=f32)
    b = np.asarray(b, dtype=f32)

    deg = (1.0 + np.bincount(row, minlength=N)).astype(f32)
    dis = (1.0 / np.sqrt(deg)).astype(f32)
    # sigma[r] = sum over messages into r (incl self) of dis[col]
    sig = np.bincount(row, weights=dis[col].astype(np.float64),
                      minlength=N).astype(f32) + dis

    disH = (dis[:, None] * H).astype(bf16)        # pre-scaled source features

    deg_i = deg.astype(np.int64)

    iota = np.tile(np.arange(P, dtype=bf16)[None, :], (P, 1))
    WTb = np.ascontiguousarray(W.T).astype(bf16)  # [d, f]
    biasB = np.tile(b[None, :], (P, 1)).astype(f32)

    bins = _pack_blocks(deg_i, CPB)               # global: 784 bins
    if bins is None:
        return None, None
    # node -> global slot: bin b is (core b//NBLK, block b%NBLK)
    order = np.argsort(bins, kind="stable")
    bin_of = bins[order]
    slot_of = np.arange(N) - np.searchsorted(bin_of, bin_of)
    node_pos = np.empty(N, dtype=np.int64)        # core*NPC + blk*P + slot
    node_pos[order] = bin_of * P + slot_of

    allr = np.concatenate([row, np.arange(N, dtype=np.int64)])  # + self loops
    allc = np.concatenate([col, np.arange(N, dtype=np.int64)])
    dst = node_pos[allr]
    ecore = dst // NPC

    in_maps = []
    for c in range(NCORES):
        em = ecore == c
        ec = allc[em]
        dstc = dst[em] - c * NPC
        dblk = dstc // P
        dk = dstc % P
        eorder = np.argsort(dblk, kind="stable")
        dblk_s = dblk[eorder]
        rank = np.arange(len(eorder)) - np.searchsorted(dblk_s, dblk_s)
        assert rank.max() < CPB * P
        cidx = dblk_s * CPB + rank // P              # chunk index
        slot = rank % P

        Hexp3 = np.zeros((NC, P, D), dtype=bf16)
        Hexp3[cidx, slot] = disH[ec[eorder]]
        HexpT = np.ascontiguousarray(
            Hexp3.transpose(1, 0, 2).reshape(P, NC * D))

        dkT = np.full((P, NC), -1.0, dtype=bf16)
        dkT[slot, cidx] = dk[eorder].astype(bf16)

        nm = (node_pos >= c * NPC) & (node_pos < (c + 1) * NPC)
        npos = node_pos[nm] - c * NPC
        sigT = np.zeros((P, NBLK), dtype=f32)
        disT = np.zeros((P, NBLK), dtype=f32)
        sigT[npos % P, npos // P] = sig[nm]
        disT[npos % P, npos // P] = dis[nm]
        # sigma as a 1-partition row for the rank-1 bias matmul:
        # sgR[0, t*P + dst] = sigma(dst, t)
        sgR = np.ascontiguousarray(
            sigT.T.reshape(1, NBLK * P)).astype(bf16)
        bR = b.reshape(1, D).astype(bf16)

        in_maps.append(dict(
            Hexp=HexpT, dkT=np.ascontiguousarray(dkT),
            sgR=sgR, bR=bR,
            disT=np.ascontiguousarray(disT),
            WTb=WTb, iota=iota,
        ))
    return in_maps, node_pos


# ------------------------------------------------------------- numpy device sim

def _sim_spmd(in_maps, CPB):
    """Numpy mirror of the device program (index-plumbing validation)."""
    import ml_dtypes
    f32 = np.float32
    bf16 = ml_dtypes.bfloat16
    outs = []
    for m in in_maps:
        Hexp = m["Hexp"].astype(f32)          # [128, NC*D]
        dkT = m["dkT"]
        iota = m["iota"].astype(f32)
        out_c = np.zeros((P, NBLK, D), dtype=f32)
        for t in range(NBLK):
            zt = np.zeros((D, P), dtype=f32)  # [d, dst]
            for j in range(CPB):
                c = t * CPB + j
                S = (iota == dkT[:, c:c + 1]).astype(f32)   # [e, dst]
                He = Hexp[:, c * D:(c + 1) * D]             # [e, d]
                zt += He.T @ S
            ztb = zt.astype(bf16).astype(f32)
            acc = (m["sgR"][0, t * P:(t + 1) * P].astype(f32)[:, None]
                   * m["bR"].astype(f32))                       # [dst, f]
            acc = acc + ztb.T @ m["WTb"].astype(f32)
            out_c[:, t, :] = np.maximum(
                acc * m["disT"][:, t:t + 1], 0.0).astype(bf16).astype(f32)
        outs.append(out_c.transpose(1, 0, 2).reshape(NPC, D))
    return outs


# ------------------------------------------------------------- device kernel

_NC_CACHE = {}
_LAST = {}          # exposes (nc, in_maps, CPB) of last kernel() call
GRP = 7             # dest blocks per DMA group (98 = 14*7)
SACT_GROUPS = 1     # trailing groups whose S builds on ACT (DVE offload)


def _build_nc(CPB):
    import concourse.bacc as bacc
    import concourse.mybir as mybir
    import concourse.tile as tile

    bf = mybir.dt.bfloat16
    f32 = mybir.dt.float32
    NC = NBLK * CPB

    nc = bacc.Bacc("TRN2", target_bir_lowering=False, debug=False,
                   num_devices=NCORES)

    Hexp = nc.dram_tensor("Hexp", [P, NC * D], bf, kind="ExternalInput").ap()
    dkT = nc.dram_tensor("dkT", [P, NC], bf, kind="ExternalInput").ap()
    sgR = nc.dram_tensor("sgR", [1, NBLK * P], bf, kind="ExternalInput").ap()
    bR = nc.dram_tensor("bR", [1, D], bf, kind="ExternalInput").ap()
    disT = nc.dram_tensor("disT", [P, NBLK], f32, kind="ExternalInput").ap()
    WTb = nc.dram_tensor("WTb", [P, D], bf, kind="ExternalInput").ap()
    iota = nc.dram_tensor("iota", [P, P], bf, kind="ExternalInput").ap()
    out = nc.dram_tensor("out", [NPC, D], bf, kind="ExternalOutput").ap()

    NG = NBLK // GRP
    with tile.TileContext(nc) as tc:
        with (
            tc.tile_pool(name="const", bufs=1) as const,
            tc.tile_pool(name="hexp", bufs=3) as hpool,
            tc.tile_pool(name="spool", bufs=2) as spool,
            tc.tile_pool(name="ztb", bufs=2) as ztbpool,
            tc.tile_pool(name="ostg", bufs=4) as opool,
            tc.tile_pool(name="zt", bufs=2, space="PSUM") as ztpool,
            tc.tile_pool(name="acc", bufs=4, space="PSUM") as accpool,
        ):
            WTb_s = const.tile([P, D], bf)
            nc.sync.dma_start(out=WTb_s[:], in_=WTb[:])
            iota7_s = const.tile([P, CPB * P], bf)
            for j in range(CPB):
                nc.sync.dma_start(out=iota7_s[:, j * P:(j + 1) * P],
                                  in_=iota[:])
            disT_s = const.tile([P, NBLK], f32)
            nc.sync.dma_start(out=disT_s[:], in_=disT[:])
            sgR_s = const.tile([1, NBLK * P], bf)
            nc.sync.dma_start(out=sgR_s[:], in_=sgR[:])
            bR_s = const.tile([1, D], bf)
            nc.sync.dma_start(out=bR_s[:], in_=bR[:])
            dkT_s = const.tile([P, NC], bf)
            nc.scalar.dma_start(out=dkT_s[:], in_=dkT[:])

            for g in range(NG):
                eng = nc.sync if g % 2 == 0 else nc.scalar
                eng2 = nc.scalar if g % 2 == 0 else nc.sync
                hx = hpool.tile([P, GRP * CPB * D], bf, tag="hx",
                                name=f"hx_{g}")
                eng.dma_start(
                    out=hx[:],
                    in_=Hexp[:, g * GRP * CPB * D:(g + 1) * GRP * CPB * D])
                ostg = opool.tile([P, GRP * D], bf, tag="o", name=f"o_{g}")

                # S for the whole group: one batched is_equal on DVE via
                # stride-0 broadcast APs (1x mode, ~6.7us per group).
                c0 = g * GRP * CPB
                S_big = spool.tile([P, GRP * CPB * P], bf, tag="s",
                                   name=f"s_{g}")
                dkb = dkT_s[:, c0:c0 + GRP * CPB].rearrange(
                    "p (l c u) -> p l c u", l=GRP,
                    u=1).broadcast_to([P, GRP, CPB, P])
                iob = iota7_s[:].rearrange(
                    "p (u c m) -> p u c m", u=1,
                    c=CPB).broadcast_to([P, GRP, CPB, P])
                nc.vector.tensor_tensor(
                    out=S_big[:].rearrange("p (l c m) -> p l c m",
                                           l=GRP, c=CPB),
                    in0=iob, in1=dkb, op=mybir.AluOpType.is_equal)

                zt_big = ztpool.tile([P, GRP * P], f32, space="PSUM",
                                     tag="zt", name=f"zt_{g}")
                for lt in range(GRP):
                    for j in range(CPB):
                        k = lt * CPB + j
                        nc.tensor.matmul(
                            out=zt_big[:, lt * P:(lt + 1) * P],
                            lhsT=hx[:, k * D:(k + 1) * D],
                            rhs=S_big[:, k * P:(k + 1) * P],
                            start=(j == 0), stop=(j == CPB - 1))
                ztb = ztbpool.tile([P, GRP * P], bf, tag="ztb")
                nc.scalar.copy(out=ztb[:], in_=zt_big[:])
                for lt in range(GRP):
                    t = g * GRP + lt
                    acc = accpool.tile([P, D], f32, space="PSUM",
                                       tag="acc", name=f"acc_{t}")
                    # rank-1 bias preload on the PE: acc = sigma^T @ b
                    nc.tensor.matmul(
                        out=acc[:], lhsT=sgR_s[:, t * P:(t + 1) * P],
                        rhs=bR_s[:], start=True, stop=False)
                    nc.tensor.matmul(
                        out=acc[:], lhsT=ztb[:, lt * P:(lt + 1) * P],
                        rhs=WTb_s[:], start=False, stop=True)
                    # relu(acc)*dis: max(.,0) then mult by dis column
                    nc.vector.tensor_scalar(
                        out=ostg[:, lt * D:(lt + 1) * D], in0=acc[:],
                        scalar1=0.0, scalar2=disT_s[:, t:t + 1],
                        op0=mybir.AluOpType.max, op1=mybir.AluOpType.mult)
                # out is partition-major (flat row = p*NBLK + t)
                eng2.dma_start(
                    out=out[:].rearrange(
                        "(p t) f -> p (t f)", p=P)[:, g * GRP * D:
                                                   (g + 1) * GRP * D],
                    in_=ostg[:])

    nc.finalize()
    return nc


def kernel(H, edge_index, W, b):
    from concourse.bass_utils import run_bass_kernel_spmd

    CPB = 7
    in_maps, node_pos = _host_prep(H, edge_index, W, b, CPB)
    if in_maps is None:
        CPB = 8
        in_maps, node_pos = _host_prep(H, edge_index, W, b, CPB)
        assert in_maps is not None

    if CPB not in _NC_CACHE:
        _NC_CACHE[CPB] = _build_nc(CPB)
    nc = _NC_CACHE[CPB]
    _LAST.update(nc=nc, in_maps=in_maps, CPB=CPB)

    res = run_bass_kernel_spmd(nc, in_maps, list(range(NCORES)))
    # device out is partition-major: flat row = p*NBLK + t -> slot (t, p)
    full = np.empty((NCORES * NPC, D), dtype=np.float32)
    for c in range(NCORES):
        o = np.asarray(res.results[c]["out"], dtype=np.float32).reshape(
            P, NBLK, D)
        full[c * NPC:(c + 1) * NPC] = o.transpose(1, 0, 2).reshape(NPC, D)
    return np.ascontiguousarray(full[node_pos])


# revision 22
# speedup vs baseline: 1.5579x; 1.5579x over previous
"""GCN layer (message passing) on 8 trn2 NeuronCores.

  out = relu(segment_sum(norm * (H@W.T + b)[col], row)),  norm = d^-1/2[row] d^-1/2[col]
  with self-loops appended; d = 1 + in-degree.

Strategy (SPMD over 8 cores, edges partitioned by destination on host):
  - Aggregate-then-transform (GCN linearity):
      out[r] = relu(dis[r] * (Z[r] @ W.T + sigma[r] * b)),
      Z[r] = sum_{e: dst=r} dis[col_e] * H[col_e],  sigma[r] = sum_e dis[col_e]
    (sums include the self-loop edge r->r).
  - Host: shard nodes contiguously (12500/core), bin-pack each core's nodes
    into 98 blocks of 128 balancing per-block message counts; lay out each
    block's messages as CPB chunks of 128 edge slots; ship pre-scaled source
    features Hexp[e] = dis[col]*H[col] (bf16) in chunk-major [slot, chunk*D]
    layout plus per-slot dest keys dkT, per-node sigT/disT, W^T, bias.
  - Device per chunk: S = (iota == dk) one-hot (DVE); zt += Hexp_chunk^T @ S
    (PE, PSUM accum over the block's CPB chunks).  Per block: ztb = bf16(zt)
    (ACT); acc = sigma*bias (DVE preload) + ztb^T... acc += ztb.T @ W^T via
    PE (start=False); out_block = relu(acc * dis) (ACT) -> DMA.
  - No collectives, no dma_gather: GPSIMD stays idle; all DMA is bulk
    contiguous HWDGE.
"""
import numpy as np

N = 100000
D = 128
NCORES = 8
P = 128
NPC_REAL = N // NCORES          # 12500 real nodes per core
NBLK = 98                       # blocks of 128 slots (12544 slots, 44 dummy)
NPC = NBLK * P                  # 12544 slots per core


# ----------------------------------------------------------------- host prep

def _pack_blocks(deg_n, cpb):
    """Bin-pack all N nodes into 784 bins (<=128 each) balancing deg sums.

    deg_n: [N] message counts (in-degree + 1).  Returns [N] bin ids with
    every bin's deg sum <= cpb*128, or None if not achievable.
    """
    cap = cpb * P
    nbins = NCORES * NBLK
    order = np.argsort(-deg_n, kind="stable")
    bins = -np.ones(len(deg_n), dtype=np.int64)
    # snake placement of descending degrees
    for r in range(0, len(order), nbins):
        chunk = order[r:r + nbins]
        ids = np.arange(len(chunk))
        if (r // nbins) % 2 == 1:
            ids = nbins - 1 - ids
        bins[chunk] = ids[:len(chunk)]
    sums = np.bincount(bins, weights=deg_n, minlength=nbins).astype(np.int64)
    cnts = np.bincount(bins, minlength=nbins)
    # greedy fixup: swap items between fullest and emptiest bins
    for _ in range(20000):
        a = int(np.argmax(sums))
        if sums[a] <= cap:
            break
        b = int(np.argmin(sums))
        need = int(sums[a] - cap)
        room = int(cap - sums[b])
        ia = np.where(bins == a)[0]
        ib = np.where(bins == b)[0]
        da, db = deg_n[ia], deg_n[ib]
        # swap (i from a, j from b): diff = da_i - db_j must be >=1 and
        # <= room; prefer the smallest diff >= need, else the largest <= room
        diff = da[:, None] - db[None, :]
        ok = (diff >= 1) & (diff <= room)
        if not ok.any():
            return None
        dd = np.where(ok, diff, -1)
        best = np.where((dd >= min(need, room)) & ok, dd,
                        np.iinfo(np.int64).max)
        if best.min() != np.iinfo(np.int64).max:
            i, j = np.unravel_index(np.argmin(best), best.shape)
        else:
            i, j = np.unravel_index(np.argmax(dd), dd.shape)
        d = int(diff[i, j])
        bins[ia[i]], bins[ib[j]] = b, a
        sums[a] -= d
        sums[b] += d
    if sums.max() > cap or cnts.max() > P:
        return None
    return bins


def _host_prep(H, edge_index, W, b, CPB):
    """Build per-core device inputs; None if CPB chunks/block don't fit."""
    import ml_dtypes
    f32 = np.float32
    bf16 = ml_dtypes.bfloat16
    NC = NBLK * CPB                  # chunks per core

    row = np.asarray(edge_index[0], dtype=np.int64)
    col = np.asarray(edge_index[1], dtype=np.int64)
    H = np.asarray(H, dtype=f32)
    W = np.asarray(W, dtype=f32)
    b = np.asarray(b, dtype=f32)

    deg = (1.0 + np.bincount(row, minlength=N)).astype(f32)
    dis = (1.0 / np.sqrt(deg)).astype(f32)
    # sigma[r] = sum over messages into r (incl self) of dis[col]
    sig = np.bincount(row, weights=dis[col].astype(np.float64),
                      minlength=N).astype(f32) + dis

    disH = (dis[:, None] * H).astype(bf16)        # pre-scaled source features

    deg_i = deg.astype(np.int64)

    iota = np.tile(np.arange(P, dtype=bf16)[None, :], (P, 1))
    WTb = np.ascontiguousarray(W.T).astype(bf16)  # [d, f]
    biasB = np.tile(b[None, :], (P, 1)).astype(f32)

    bins = _pack_blocks(deg_i, CPB)               # global: 784 bins
    if bins is None:
        return None, None
    # node -> global slot: bin b is (core b//NBLK, block b%NBLK)
    order = np.argsort(bins, kind="stable")
    bin_of = bins[order]
    slot_of = np.arange(N) - np.searchsorted(bin_of, bin_of)
    node_pos = np.empty(N, dtype=np.int64)        # core*NPC + blk*P + slot
    node_pos[order] = bin_of * P + slot_of

    allr = np.concatenate([row, np.arange(N, dtype=np.int64)])  # + self loops
    allc = np.concatenate([col, np.arange(N, dtype=np.int64)])
    dst = node_pos[allr]
    ecore = dst // NPC

    in_maps = []
    for c in range(NCORES):
        em = ecore == c
        ec = allc[em]
        dstc = dst[em] - c * NPC
        dblk = dstc // P
        dk = dstc % P
        eorder = np.argsort(dblk, kind="stable")
        dblk_s = dblk[eorder]
        rank = np.arange(len(eorder)) - np.searchsorted(dblk_s, dblk_s)
        assert rank.max() < CPB * P
        cidx = dblk_s * CPB + rank // P              # chunk index
        slot = rank % P

        Hexp3 = np.zeros((NC, P, D), dtype=bf16)
        Hexp3[cidx, slot] = disH[ec[eorder]]
        HexpT = np.ascontiguousarray(
            Hexp3.transpose(1, 0, 2).reshape(P, NC * D))

        dkT = np.full((P, NC), -1.0, dtype=bf16)
        dkT[slot, cidx] = dk[eorder].astype(bf16)
        # local_scatter indices: for block t, slot p scatters ones into
        # positions j*128 + dk (one per chunk j), -1 padded to 8 indices
        lsI = np.full((P, NBLK, 8), -1, dtype=np.int16)
        lsI[slot, cidx // CPB, cidx % CPB] = ((cidx % CPB) * P
                                              + dk[eorder]).astype(np.int16)
        lsI = np.ascontiguousarray(lsI.reshape(P, NBLK * 8))
        onesD = np.ones((P, 8), dtype=bf16)

        nm = (node_pos >= c * NPC) & (node_pos < (c + 1) * NPC)
        npos = node_pos[nm] - c * NPC
        sigT = np.zeros((P, NBLK), dtype=f32)
        disT = np.zeros((P, NBLK), dtype=f32)
        sigT[npos % P, npos // P] = sig[nm]
        disT[npos % P, npos // P] = dis[nm]
        # sigma as a 1-partition row for the rank-1 bias matmul:
        # sgR[0, t*P + dst] = sigma(dst, t)
        sgR = np.ascontiguousarray(
            sigT.T.reshape(1, NBLK * P)).astype(bf16)
        bR = b.reshape(1, D).astype(bf16)

        in_maps.append(dict(
            Hexp=HexpT, dkT=np.ascontiguousarray(dkT),
            lsI=lsI, onesD=onesD,
            sgR=sgR, bR=bR,
            disT=np.ascontiguousarray(disT),
            WTb=WTb, iota=iota,
        ))
    return in_maps, node_pos


# ------------------------------------------------------------- numpy device sim

def _sim_spmd(in_maps, CPB):
    """Numpy mirror of the device program (index-plumbing validation)."""
    import ml_dtypes
    f32 = np.float32
    bf16 = ml_dtypes.bfloat16
    outs = []
    for m in in_maps:
        Hexp = m["Hexp"].astype(f32)          # [128, NC*D]
        dkT = m["dkT"]
        iota = m["iota"].astype(f32)
        out_c = np.zeros((P, NBLK, D), dtype=f32)
        for t in range(NBLK):
            zt = np.zeros((D, P), dtype=f32)  # [d, dst]
            for j in range(CPB):
                c = t * CPB + j
                S = (iota == dkT[:, c:c + 1]).astype(f32)   # [e, dst]
                He = Hexp[:, c * D:(c + 1) * D]             # [e, d]
                zt += He.T @ S
            ztb = zt.astype(bf16).astype(f32)
            acc = (m["sgR"][0, t * P:(t + 1) * P].astype(f32)[:, None]
                   * m["bR"].astype(f32))                       # [dst, f]
            acc = acc + ztb.T @ m["WTb"].astype(f32)
            out_c[:, t, :] = np.maximum(
                acc * m["disT"][:, t:t + 1], 0.0).astype(bf16).astype(f32)
        outs.append(out_c.transpose(1, 0, 2).reshape(NPC, D))
    return outs


# ------------------------------------------------------------- device kernel

_NC_CACHE = {}
_LAST = {}          # exposes (nc, in_maps, CPB) of last kernel() call
GRP = 7             # dest blocks per DMA group (98 = 14*7)
SACT_GROUPS = 1     # trailing groups whose S builds on ACT (DVE offload)


def _build_nc(CPB):
    import concourse.bacc as bacc
    import concourse.mybir as mybir
    import concourse.tile as tile

    bf = mybir.dt.bfloat16
    f32 = mybir.dt.float32
    NC = NBLK * CPB

    nc = bacc.Bacc("TRN2", target_bir_lowering=False, debug=False,
                   num_devices=NCORES)

    Hexp = nc.dram_tensor("Hexp", [P, NC * D], bf, kind="ExternalInput").ap()
    dkT = nc.dram_tensor("dkT", [P, NC], bf, kind="ExternalInput").ap()
    lsI = nc.dram_tensor("lsI", [P, NBLK * 8], mybir.dt.int16,
                         kind="ExternalInput").ap()
    onesD = nc.dram_tensor("onesD", [P, 8], bf, kind="ExternalInput").ap()
    sgR = nc.dram_tensor("sgR", [1, NBLK * P], bf, kind="ExternalInput").ap()
    bR = nc.dram_tensor("bR", [1, D], bf, kind="ExternalInput").ap()
    disT = nc.dram_tensor("disT", [P, NBLK], f32, kind="ExternalInput").ap()
    WTb = nc.dram_tensor("WTb", [P, D], bf, kind="ExternalInput").ap()
    iota = nc.dram_tensor("iota", [P, P], bf, kind="ExternalInput").ap()
    out = nc.dram_tensor("out", [NPC, D], bf, kind="ExternalOutput").ap()

    NG = NBLK // GRP
    with tile.TileContext(nc) as tc:
        with (
            tc.tile_pool(name="const", bufs=1) as const,
            tc.tile_pool(name="hexp", bufs=3) as hpool,
            tc.tile_pool(name="spool", bufs=6) as spool,
            tc.tile_pool(name="ztb", bufs=2) as ztbpool,
            tc.tile_pool(name="ostg", bufs=4) as opool,
            tc.tile_pool(name="zt", bufs=2, space="PSUM") as ztpool,
            tc.tile_pool(name="acc", bufs=4, space="PSUM") as accpool,
        ):
            from concourse import library_config
            nc.gpsimd.load_library(library_config.local_scatter)

            WTb_s = const.tile([P, D], bf)
            nc.sync.dma_start(out=WTb_s[:], in_=WTb[:])
            iota7_s = const.tile([P, CPB * P], bf)
            for j in range(CPB):
                nc.sync.dma_start(out=iota7_s[:, j * P:(j + 1) * P],
                                  in_=iota[:])
            disT_s = const.tile([P, NBLK], f32)
            nc.sync.dma_start(out=disT_s[:], in_=disT[:])
            sgR_s = const.tile([1, NBLK * P], bf)
            nc.sync.dma_start(out=sgR_s[:], in_=sgR[:])
            bR_s = const.tile([1, D], bf)
            nc.sync.dma_start(out=bR_s[:], in_=bR[:])
            dkT_s = const.tile([P, NC], bf)
            nc.scalar.dma_start(out=dkT_s[:], in_=dkT[:])
            lsI_s = const.tile([P, NBLK * 8], mybir.dt.int16)
            nc.scalar.dma_start(out=lsI_s[:], in_=lsI[:])
            onesD_s = const.tile([P, 8], bf)
            nc.scalar.dma_start(out=onesD_s[:], in_=onesD[:])

            # dedicated DMA queues: loads on sync, stores on scalar, so the
            # hx prefetch never queues behind an out store
            for g in range(NG):
                hx = hpool.tile([P, GRP * CPB * D], bf, tag="hx",
                                name=f"hx_{g}")
                nc.sync.dma_start(
                    out=hx[:],
                    in_=Hexp[:, g * GRP * CPB * D:(g + 1) * GRP * CPB * D])
                ostg = opool.tile([P, GRP * D], bf, tag="o", name=f"o_{g}")

                zt_big = ztpool.tile([P, GRP * P], f32, space="PSUM",
                                     tag="zt", name=f"zt_{g}")
                for lt in range(GRP):
                    t = g * GRP + lt
                    # S for this block: one-hot columns, built on DVE
                    # (is_equal with stride-0 broadcast) or GPSIMD
                    # (local_scatter ucode), alternating to split the load.
                    S7 = spool.tile([P, CPB * P], bf, tag="s",
                                    name=f"s_{t}")
                    if t % 2 == 0:
                        nc.gpsimd.local_scatter(
                            S7[:], onesD_s[:],
                            lsI_s[:, t * 8:(t + 1) * 8],
                            channels=P, num_elems=CPB * P, num_idxs=8)
                    else:
                        c0 = t * CPB
                        dkb = dkT_s[:, c0:c0 + CPB].rearrange(
                            "p (c u) -> p c u",
                            u=1).broadcast_to([P, CPB, P])
                        nc.vector.tensor_tensor(
                            out=S7[:].rearrange("p (c m) -> p c m", c=CPB),
                            in0=iota7_s[:].rearrange("p (c m) -> p c m",
                                                     c=CPB),
                            in1=dkb, op=mybir.AluOpType.is_equal)
                    for j in range(CPB):
                        nc.tensor.matmul(
                            out=zt_big[:, lt * P:(lt + 1) * P],
                            lhsT=hx[:, (lt * CPB + j) * D:
                                    (lt * CPB + j + 1) * D],
                            rhs=S7[:, j * P:(j + 1) * P],
                            start=(j == 0), stop=(j == CPB - 1))
                ztb = ztbpool.tile([P, GRP * P], bf, tag="ztb")
                nc.scalar.copy(out=ztb[:], in_=zt_big[:])
                for lt in range(GRP):
                    t = g * GRP + lt
                    acc = accpool.tile([P, D], f32, space="PSUM",
                                       tag="acc", name=f"acc_{t}")
                    # rank-1 bias preload on the PE: acc = sigma^T @ b
                    nc.tensor.matmul(
                        out=acc[:], lhsT=sgR_s[:, t * P:(t + 1) * P],
                        rhs=bR_s[:], start=True, stop=False)
                    nc.tensor.matmul(
                        out=acc[:], lhsT=ztb[:, lt * P:(lt + 1) * P],
                        rhs=WTb_s[:], start=False, stop=True)
                    # relu(acc)*dis on ACT: func applies after scale
                    nc.scalar.activation(
                        out=ostg[:, lt * D:(lt + 1) * D], in_=acc[:],
                        func=mybir.ActivationFunctionType.Relu,
                        scale=disT_s[:, t:t + 1])
                # out is partition-major (flat row = p*NBLK + t)
                nc.scalar.dma_start(
                    out=out[:].rearrange(
                        "(p t) f -> p (t f)", p=P)[:, g * GRP * D:
                                                   (g + 1) * GRP * D],
                    in_=ostg[:])

    nc.finalize()
    return nc


def kernel(H, edge_index, W, b):
    from concourse.bass_utils import run_bass_kernel_spmd

    CPB = 7
    in_maps, node_pos = _host_prep(H, edge_index, W, b, CPB)
    if in_maps is None:
        CPB = 8
        in_maps, node_pos = _host_prep(H, edge_index, W, b, CPB)
        assert in_maps is not None

    if CPB not in _NC_CACHE:
        _NC_CACHE[CPB] = _build_nc(CPB)
    nc = _NC_CACHE[CPB]
    _LAST.update(nc=nc, in_maps=in_maps, CPB=CPB)

    res = run_bass_kernel_spmd(nc, in_maps, list(range(NCORES)))
    # device out is partition-major: flat row = p*NBLK + t -> slot (t, p)
    full = np.empty((NCORES * NPC, D), dtype=np.float32)
    for c in range(NCORES):
        o = np.asarray(res.results[c]["out"], dtype=np.float32).reshape(
            P, NBLK, D)
        full[c * NPC:(c + 1) * NPC] = o.transpose(1, 0, 2).reshape(NPC, D)
    return np.ascontiguousarray(full[node_pos])


# revision 23
# speedup vs baseline: 1.6102x; 1.0336x over previous
"""GCN layer (message passing) on 8 trn2 NeuronCores.

  out = relu(segment_sum(norm * (H@W.T + b)[col], row)),  norm = d^-1/2[row] d^-1/2[col]
  with self-loops appended; d = 1 + in-degree.

Strategy (SPMD over 8 cores, edges partitioned by destination on host):
  - Aggregate-then-transform (GCN linearity):
      out[r] = relu(dis[r] * (Z[r] @ W.T + sigma[r] * b)),
      Z[r] = sum_{e: dst=r} dis[col_e] * H[col_e],  sigma[r] = sum_e dis[col_e]
    (sums include the self-loop edge r->r).
  - Host: shard nodes contiguously (12500/core), bin-pack each core's nodes
    into 98 blocks of 128 balancing per-block message counts; lay out each
    block's messages as CPB chunks of 128 edge slots; ship pre-scaled source
    features Hexp[e] = dis[col]*H[col] (bf16) in chunk-major [slot, chunk*D]
    layout plus per-slot dest keys dkT, per-node sigT/disT, W^T, bias.
  - Device per chunk: S = (iota == dk) one-hot (DVE); zt += Hexp_chunk^T @ S
    (PE, PSUM accum over the block's CPB chunks).  Per block: ztb = bf16(zt)
    (ACT); acc = sigma*bias (DVE preload) + ztb^T... acc += ztb.T @ W^T via
    PE (start=False); out_block = relu(acc * dis) (ACT) -> DMA.
  - No collectives, no dma_gather: GPSIMD stays idle; all DMA is bulk
    contiguous HWDGE.
"""
import numpy as np

N = 100000
D = 128
NCORES = 8
P = 128
NPC_REAL = N // NCORES          # 12500 real nodes per core
NBLK = 98                       # blocks of 128 slots (12544 slots, 44 dummy)
NPC = NBLK * P                  # 12544 slots per core


# ----------------------------------------------------------------- host prep

def _pack_blocks(deg_n, cpb):
    """Bin-pack all N nodes into 784 bins (<=128 each) balancing deg sums.

    deg_n: [N] message counts (in-degree + 1).  Returns [N] bin ids with
    every bin's deg sum <= cpb*128, or None if not achievable.
    """
    cap = cpb * P
    nbins = NCORES * NBLK
    order = np.argsort(-deg_n, kind="stable")
    bins = -np.ones(len(deg_n), dtype=np.int64)
    # snake placement of descending degrees
    for r in range(0, len(order), nbins):
        chunk = order[r:r + nbins]
        ids = np.arange(len(chunk))
        if (r // nbins) % 2 == 1:
            ids = nbins - 1 - ids
        bins[chunk] = ids[:len(chunk)]
    sums = np.bincount(bins, weights=deg_n, minlength=nbins).astype(np.int64)
    cnts = np.bincount(bins, minlength=nbins)
    # greedy fixup: swap items between fullest and emptiest bins
    for _ in range(20000):
        a = int(np.argmax(sums))
        if sums[a] <= cap:
            break
        b = int(np.argmin(sums))
        need = int(sums[a] - cap)
        room = int(cap - sums[b])
        ia = np.where(bins == a)[0]
        ib = np.where(bins == b)[0]
        da, db = deg_n[ia], deg_n[ib]
        # swap (i from a, j from b): diff = da_i - db_j must be >=1 and
        # <= room; prefer the smallest diff >= need, else the largest <= room
        diff = da[:, None] - db[None, :]
        ok = (diff >= 1) & (diff <= room)
        if not ok.any():
            return None
        dd = np.where(ok, diff, -1)
        best = np.where((dd >= min(need, room)) & ok, dd,
                        np.iinfo(np.int64).max)
        if best.min() != np.iinfo(np.int64).max:
            i, j = np.unravel_index(np.argmin(best), best.shape)
        else:
            i, j = np.unravel_index(np.argmax(dd), dd.shape)
        d = int(diff[i, j])
        bins[ia[i]], bins[ib[j]] = b, a
        sums[a] -= d
        sums[b] += d
    if sums.max() > cap or cnts.max() > P:
        return None
    return bins


def _host_prep(H, edge_index, W, b, CPB):
    """Build per-core device inputs; None if CPB chunks/block don't fit."""
    import ml_dtypes
    f32 = np.float32
    bf16 = ml_dtypes.bfloat16
    NC = NBLK * CPB                  # chunks per core

    row = np.asarray(edge_index[0], dtype=np.int64)
    col = np.asarray(edge_index[1], dtype=np.int64)
    H = np.asarray(H, dtype=f32)
    W = np.asarray(W, dtype=f32)
    b = np.asarray(b, dtype=f32)

    deg = (1.0 + np.bincount(row, minlength=N)).astype(f32)
    dis = (1.0 / np.sqrt(deg)).astype(f32)
    # sigma[r] = sum over messages into r (incl self) of dis[col]
    sig = np.bincount(row, weights=dis[col].astype(np.float64),
                      minlength=N).astype(f32) + dis

    disH = (dis[:, None] * H).astype(bf16)        # pre-scaled source features

    deg_i = deg.astype(np.int64)

    iota = np.tile(np.arange(P, dtype=bf16)[None, :], (P, 1))
    WTb = np.ascontiguousarray(W.T).astype(bf16)  # [d, f]
    biasB = np.tile(b[None, :], (P, 1)).astype(f32)

    bins = _pack_blocks(deg_i, CPB)               # global: 784 bins
    if bins is None:
        return None, None
    # node -> global slot: bin b is (core b//NBLK, block b%NBLK)
    order = np.argsort(bins, kind="stable")
    bin_of = bins[order]
    slot_of = np.arange(N) - np.searchsorted(bin_of, bin_of)
    node_pos = np.empty(N, dtype=np.int64)        # core*NPC + blk*P + slot
    node_pos[order] = bin_of * P + slot_of

    allr = np.concatenate([row, np.arange(N, dtype=np.int64)])  # + self loops
    allc = np.concatenate([col, np.arange(N, dtype=np.int64)])
    dst = node_pos[allr]
    ecore = dst // NPC

    in_maps = []
    for c in range(NCORES):
        em = ecore == c
        ec = allc[em]
        dstc = dst[em] - c * NPC
        dblk = dstc // P
        dk = dstc % P
        eorder = np.argsort(dblk, kind="stable")
        dblk_s = dblk[eorder]
        rank = np.arange(len(eorder)) - np.searchsorted(dblk_s, dblk_s)
        assert rank.max() < CPB * P
        cidx = dblk_s * CPB + rank // P              # chunk index
        slot = rank % P

        Hexp3 = np.zeros((NC, P, D), dtype=bf16)
        Hexp3[cidx, slot] = disH[ec[eorder]]
        HexpT = np.ascontiguousarray(
            Hexp3.transpose(1, 0, 2).reshape(P, NC * D))

        dkT = np.full((P, NC), -1.0, dtype=bf16)
        dkT[slot, cidx] = dk[eorder].astype(bf16)
        # local_scatter indices: for block t, slot p scatters ones into
        # positions j*128 + dk (one per chunk j), -1 padded to 8 indices
        lsI = np.full((P, NBLK, 8), -1, dtype=np.int16)
        lsI[slot, cidx // CPB, cidx % CPB] = ((cidx % CPB) * P
                                              + dk[eorder]).astype(np.int16)
        lsI = np.ascontiguousarray(lsI.reshape(P, NBLK * 8))
        onesD = np.ones((P, 8), dtype=bf16)

        nm = (node_pos >= c * NPC) & (node_pos < (c + 1) * NPC)
        npos = node_pos[nm] - c * NPC
        sigT = np.zeros((P, NBLK), dtype=f32)
        disT = np.zeros((P, NBLK), dtype=f32)
        sigT[npos % P, npos // P] = sig[nm]
        disT[npos % P, npos // P] = dis[nm]
        # sigma as a 1-partition row for the rank-1 bias matmul:
        # sgR[0, t*P + dst] = sigma(dst, t)
        sgR = np.ascontiguousarray(
            sigT.T.reshape(1, NBLK * P)).astype(bf16)
        bR = b.reshape(1, D).astype(bf16)

        in_maps.append(dict(
            Hexp=HexpT, dkT=np.ascontiguousarray(dkT),
            lsI=lsI, onesD=onesD,
            sgR=sgR, bR=bR,
            disT=np.ascontiguousarray(disT),
            WTb=WTb, iota=iota,
        ))
    return in_maps, node_pos


# ------------------------------------------------------------- numpy device sim

def _sim_spmd(in_maps, CPB):
    """Numpy mirror of the device program (index-plumbing validation)."""
    import ml_dtypes
    f32 = np.float32
    bf16 = ml_dtypes.bfloat16
    outs = []
    for m in in_maps:
        Hexp = m["Hexp"].astype(f32)          # [128, NC*D]
        dkT = m["dkT"]
        iota = m["iota"].astype(f32)
        out_c = np.zeros((P, NBLK, D), dtype=f32)
        for t in range(NBLK):
            zt = np.zeros((D, P), dtype=f32)  # [d, dst]
            for j in range(CPB):
                c = t * CPB + j
                S = (iota == dkT[:, c:c + 1]).astype(f32)   # [e, dst]
                He = Hexp[:, c * D:(c + 1) * D]             # [e, d]
                zt += He.T @ S
            ztb = zt.astype(bf16).astype(f32)
            acc = (m["sgR"][0, t * P:(t + 1) * P].astype(f32)[:, None]
                   * m["bR"].astype(f32))                       # [dst, f]
            acc = acc + ztb.T @ m["WTb"].astype(f32)
            out_c[:, t, :] = np.maximum(
                acc * m["disT"][:, t:t + 1], 0.0).astype(bf16).astype(f32)
        outs.append(out_c.transpose(1, 0, 2).reshape(NPC, D))
    return outs


# ------------------------------------------------------------- device kernel

_NC_CACHE = {}
_LAST = {}          # exposes (nc, in_maps, CPB) of last kernel() call
GRP = 7             # dest blocks per DMA group (98 = 14*7)
SACT_GROUPS = 1     # trailing groups whose S builds on ACT (DVE offload)


def _build_nc(CPB):
    import concourse.bacc as bacc
    import concourse.mybir as mybir
    import concourse.tile as tile

    bf = mybir.dt.bfloat16
    f32 = mybir.dt.float32
    NC = NBLK * CPB

    nc = bacc.Bacc("TRN2", target_bir_lowering=False, debug=False,
                   num_devices=NCORES)

    Hexp = nc.dram_tensor("Hexp", [P, NC * D], bf, kind="ExternalInput").ap()
    dkT = nc.dram_tensor("dkT", [P, NC], bf, kind="ExternalInput").ap()
    lsI = nc.dram_tensor("lsI", [P, NBLK * 8], mybir.dt.int16,
                         kind="ExternalInput").ap()
    onesD = nc.dram_tensor("onesD", [P, 8], bf, kind="ExternalInput").ap()
    sgR = nc.dram_tensor("sgR", [1, NBLK * P], bf, kind="ExternalInput").ap()
    bR = nc.dram_tensor("bR", [1, D], bf, kind="ExternalInput").ap()
    disT = nc.dram_tensor("disT", [P, NBLK], f32, kind="ExternalInput").ap()
    WTb = nc.dram_tensor("WTb", [P, D], bf, kind="ExternalInput").ap()
    iota = nc.dram_tensor("iota", [P, P], bf, kind="ExternalInput").ap()
    out = nc.dram_tensor("out", [NPC, D], bf, kind="ExternalOutput").ap()

    NG = NBLK // GRP
    with tile.TileContext(nc) as tc:
        with (
            tc.tile_pool(name="const", bufs=1) as const,
            tc.tile_pool(name="hexp", bufs=3) as hpool,
            tc.tile_pool(name="spool", bufs=6) as spool,
            tc.tile_pool(name="ztb", bufs=2) as ztbpool,
            tc.tile_pool(name="ostg", bufs=4) as opool,
            tc.tile_pool(name="zt", bufs=2, space="PSUM") as ztpool,
            tc.tile_pool(name="acc", bufs=4, space="PSUM") as accpool,
        ):
            from concourse import library_config
            nc.gpsimd.load_library(library_config.local_scatter)

            WTb_s = const.tile([P, D], bf)
            nc.sync.dma_start(out=WTb_s[:], in_=WTb[:])
            iota7_s = const.tile([P, CPB * P], bf)
            for j in range(CPB):
                nc.sync.dma_start(out=iota7_s[:, j * P:(j + 1) * P],
                                  in_=iota[:])
            disT_s = const.tile([P, NBLK], f32)
            nc.sync.dma_start(out=disT_s[:], in_=disT[:])
            sgR_s = const.tile([1, NBLK * P], bf)
            nc.sync.dma_start(out=sgR_s[:], in_=sgR[:])
            bR_s = const.tile([1, D], bf)
            nc.sync.dma_start(out=bR_s[:], in_=bR[:])
            dkT_s = const.tile([P, NC], bf)
            nc.scalar.dma_start(out=dkT_s[:], in_=dkT[:])
            lsI_s = const.tile([P, NBLK * 8], mybir.dt.int16)
            nc.scalar.dma_start(out=lsI_s[:], in_=lsI[:])
            onesD_s = const.tile([P, 8], bf)
            nc.scalar.dma_start(out=onesD_s[:], in_=onesD[:])

            # dedicated DMA queues: loads on sync, stores on scalar, so the
            # hx prefetch never queues behind an out store.
            # Software pipeline: group g's W-stage (which waits on the ACT
            # ztb copy) is emitted AFTER group g+1's chunk matmuls, so the
            # in-order PE queue never stalls on the copy.
            state = {}

            def emit_front(g):
                hx = hpool.tile([P, GRP * CPB * D], bf, tag="hx",
                                name=f"hx_{g}")
                nc.sync.dma_start(
                    out=hx[:],
                    in_=Hexp[:, g * GRP * CPB * D:(g + 1) * GRP * CPB * D])
                zt_big = ztpool.tile([P, GRP * P], f32, space="PSUM",
                                     tag="zt", name=f"zt_{g}")
                for lt in range(GRP):
                    t = g * GRP + lt
                    # S: one-hot columns, built on GPSIMD (local_scatter
                    # ucode) or DVE (is_equal + stride-0 broadcast),
                    # alternating to split the load.
                    S7 = spool.tile([P, CPB * P], bf, tag="s",
                                    name=f"s_{t}")
                    if t % 2 == 0:
                        nc.gpsimd.local_scatter(
                            S7[:], onesD_s[:],
                            lsI_s[:, t * 8:(t + 1) * 8],
                            channels=P, num_elems=CPB * P, num_idxs=8)
                    else:
                        c0 = t * CPB
                        dkb = dkT_s[:, c0:c0 + CPB].rearrange(
                            "p (c u) -> p c u",
                            u=1).broadcast_to([P, CPB, P])
                        nc.vector.tensor_tensor(
                            out=S7[:].rearrange("p (c m) -> p c m", c=CPB),
                            in0=iota7_s[:].rearrange("p (c m) -> p c m",
                                                     c=CPB),
                            in1=dkb, op=mybir.AluOpType.is_equal)
                    for j in range(CPB):
                        nc.tensor.matmul(
                            out=zt_big[:, lt * P:(lt + 1) * P],
                            lhsT=hx[:, (lt * CPB + j) * D:
                                    (lt * CPB + j + 1) * D],
                            rhs=S7[:, j * P:(j + 1) * P],
                            start=(j == 0), stop=(j == CPB - 1))
                ztb = ztbpool.tile([P, GRP * P], bf, tag="ztb",
                                   name=f"ztb_{g}")
                nc.scalar.copy(out=ztb[:], in_=zt_big[:])
                state[g] = ztb

            def emit_back(g):
                ztb = state.pop(g)
                ostg = opool.tile([P, GRP * D], bf, tag="o", name=f"o_{g}")
                for lt in range(GRP):
                    t = g * GRP + lt
                    acc = accpool.tile([P, D], f32, space="PSUM",
                                       tag="acc", name=f"acc_{t}")
                    # rank-1 bias preload on the PE: acc = sigma^T @ b
                    nc.tensor.matmul(
                        out=acc[:], lhsT=sgR_s[:, t * P:(t + 1) * P],
                        rhs=bR_s[:], start=True, stop=False)
                    nc.tensor.matmul(
                        out=acc[:], lhsT=ztb[:, lt * P:(lt + 1) * P],
                        rhs=WTb_s[:], start=False, stop=True)
                    # relu(acc)*dis on ACT: func applies after scale
                    nc.scalar.activation(
                        out=ostg[:, lt * D:(lt + 1) * D], in_=acc[:],
                        func=mybir.ActivationFunctionType.Relu,
                        scale=disT_s[:, t:t + 1])
                # out is partition-major (flat row = p*NBLK + t)
                nc.scalar.dma_start(
                    out=out[:].rearrange(
                        "(p t) f -> p (t f)", p=P)[:, g * GRP * D:
                                                   (g + 1) * GRP * D],
                    in_=ostg[:])

            for g in range(NG + 1):
                if g < NG:
                    emit_front(g)
                if g >= 1:
                    emit_back(g - 1)

    nc.finalize()
    return nc


def kernel(H, edge_index, W, b):
    from concourse.bass_utils import run_bass_kernel_spmd

    CPB = 7
    in_maps, node_pos = _host_prep(H, edge_index, W, b, CPB)
    if in_maps is None:
        CPB = 8
        in_maps, node_pos = _host_prep(H, edge_index, W, b, CPB)
        assert in_maps is not None

    if CPB not in _NC_CACHE:
        _NC_CACHE[CPB] = _build_nc(CPB)
    nc = _NC_CACHE[CPB]
    _LAST.update(nc=nc, in_maps=in_maps, CPB=CPB)

    res = run_bass_kernel_spmd(nc, in_maps, list(range(NCORES)))
    # device out is partition-major: flat row = p*NBLK + t -> slot (t, p)
    full = np.empty((NCORES * NPC, D), dtype=np.float32)
    for c in range(NCORES):
        o = np.asarray(res.results[c]["out"], dtype=np.float32).reshape(
            P, NBLK, D)
        full[c * NPC:(c + 1) * NPC] = o.transpose(1, 0, 2).reshape(NPC, D)
    return np.ascontiguousarray(full[node_pos])
